# revision 13
# baseline (speedup 1.0000x reference)
"""Trainium2 Bass kernel for nn_Model_17085379903564 (HiPPO-LegT multiscale
spectral forecaster).

Math: the reference normalizes x per (b,e) series, runs a HiPPO-LegT scan,
takes 32 rFFT modes of the state trajectory, mixes modes with complex
weights w, evaluates the irFFT at t=511, projects on Legendre polynomials
(Em), mixes two scales with an MLP, and un-normalizes.

Everything from the input to the Legendre projection is LINEAR with
constant coefficients, so per scale (L = 512 or 1024):

  Exf[be, (n,k)] = sum_t f[t,be] * W2[t,(n,k)]        (one dense operator)
  xdc[be,o]      = sum_(n,k) Re(Exf).Re(w) - Im(Exf).Im(w)
  dec            = xdc @ Em[-512:].T

W2 folds the scan kernel, DFT, and point-irFFT weights; it is numerically
low rank, so we factor W2 ~= U @ V by SVD with rank 64 (rel err ~1.7e-3
end to end vs the 2e-2 gate; the error is dominated by bf16 series
statistics, not the spectral path).  Per core (n-shard of V/w):

  gT  = U.T @ f - SU x mu     (fp8 DoubleRow; the PE stationary operand is
                               the U chunk so gT needs no transpose; the
                               rank-1 term is the instance-norm correction)
  P   = V @ w                 (fp8 DoubleRow, contraction 2048)
  Q   = P @ EmT               (P cast to fp8, PE-transposed, one DoubleRow
                               matmul per scale over the o=256 contraction)
  dec += gT_s.T @ (m_s Q_s)   (both scales into one PSUM; m_s folds the
                               mlp weight and every fp8 scale factor)

All fp8 scale factors (w absmax, V/U/EmT/x ranges) are folded into the
host-built mlpwb vector; the un-norm std cancels algebraically and only
the affine bias b*std + mu remains, applied on DVE while storing.

Sharding (8 cores): V/w sharded over the spectral dim n (32 of 256 rows
per core) -> per-core partial dec; host sums the 8 fp16 partials.
All DRAM operands are host-packed in exact SBUF layout ([128, X]
contiguous) so every DMA is descriptor-cheap.
"""

from contextlib import ExitStack

import ml_dtypes
import numpy as np

import concourse.bacc as bacc
import concourse.bass as bass
import concourse.mybir as mybir
import concourse.tile as tile
from concourse.bass_utils import run_bass_kernel_spmd
from concourse.masks import make_identity

# ---- problem constants (hardcoded; kernel.py must be self-contained) ----
B_SZ = 4
SEQ_LEN = 1024
PRED_LEN = 512
E_IN = 32
N_ORD = 256
MODES = 32
MULTISCALE = (1, 2)
BE = B_SZ * E_IN            # 128
N_CORES = 8
NSL = N_ORD // N_CORES      # 32  n-rows per core
NK = 2 * NSL * MODES        # 2048 contraction length per core (re+im)
NKC = NK // 256             # 8   DoubleRow chunks (256 contraction each)
RANK = 64                   # SVD rank kept per scale

FSC = 32.0                  # fp8 scale for x (N(0,1) -> max ~128)
VSCK = 1.5e-3               # extra V shrink so P lands in fp8 range

F32 = mybir.dt.float32
F16 = mybir.dt.float16
BF16 = mybir.dt.bfloat16
FP8 = mybir.dt.float8e4
BF16_NP = np.dtype(ml_dtypes.bfloat16)
FP8_NP = np.dtype(ml_dtypes.float8_e4m3)


def _f8(a):
    return np.clip(a, -240.0, 240.0).astype(FP8_NP)


# ---------------------------------------------------------------- constants
def _transition_lmu(N):
    Q = np.arange(N, dtype=np.float64)
    R = (2 * Q + 1)[:, None]
    j, i = np.meshgrid(Q, Q)
    A = np.where(i < j, -1.0, (-1.0) ** (i - j + 1)) * R
    Bv = ((-1.0) ** Q[:, None] * R)[:, 0]
    return A, Bv


def _bilinear(A, Bv, dt):
    I = np.eye(A.shape[0])
    M = I - (dt / 2.0) * A
    Ad = np.linalg.solve(M, I + (dt / 2.0) * A)
    Bd = np.linalg.solve(M, dt * Bv)
    return Ad, Bd


def _legendre_vander(x, N):
    P = np.zeros((N, x.shape[0]))
    P[0] = 1.0
    if N > 1:
        P[1] = x
    for n in range(1, N - 1):
        P[n + 1] = ((2 * n + 1) * x * P[n] - n * P[n - 1]) / (n + 1)
    return P.T


def _scale_consts(ms):
    """Per-scale constants: U (L, r) f32 (sv and 1/vsc folded in),
    Vn (r, 2*N*MODES) f32 scaled to fp8 range, EmT (N_ORD, PRED_LEN)."""
    L = ms * PRED_LEN
    A, Bv = _transition_lmu(N_ORD)
    Ad, Bd = _bilinear(A, Bv, 1.0 / L)
    vals = np.arange(0.0, 1.0, 1.0 / L)
    Em = _legendre_vander(1.0 - 2.0 * vals, N_ORD)        # (L, N)

    G = np.empty((L, N_ORD))
    g = Bd.copy()
    for m in range(L):
        G[m] = g
        g = Ad @ g
    k = np.arange(MODES)
    z = np.exp(-2j * np.pi * k / L)                       # (32,)
    zm = z[None, :] ** np.arange(L)[:, None]              # (L, 32)
    Gpre = np.cumsum(zm[:, None, :] * G[:, :, None], axis=0)   # (L, N, 32)
    W = zm[:, None, :] * Gpre[::-1]                       # (L, N, 32) complex
    e = (2.0 - (k == 0)) / L * np.exp(2j * np.pi * k * (PRED_LEN - 1) / L)
    W2 = W * e[None, None, :]

    M = np.concatenate(
        [W2.real.reshape(L, -1), (-W2.imag).reshape(L, -1)], axis=1)
    # economy SVD via the (L, L) Gram matrix: far faster than svd(M) and
    # numerically fine for the top-64 subspace kept here
    evals, evecs = np.linalg.eigh(M @ M.T)
    order = np.argsort(evals)[::-1][:RANK]
    sv = np.sqrt(np.maximum(evals[order], 0.0))
    Uf = evecs[:, order]                                  # (L, r)
    V = (Uf.T @ M) / sv[:, None]                          # (r, 16384) unit rows
    vsc = 128.0 / np.abs(V).max()
    U = Uf * (sv / vsc)                                   # (L, r)
    Vn = V * vsc                                          # max abs 128
    return (np.ascontiguousarray(U.astype(np.float32)),
            np.ascontiguousarray(Vn.astype(np.float32)),
            np.ascontiguousarray(Em[-PRED_LEN:].T.astype(np.float32)))


_CONSTS = None


def _get_consts():
    global _CONSTS
    if _CONSTS is None:
        _CONSTS = [_scale_consts(ms) for ms in MULTISCALE]
    return _CONSTS


# ---------------------------------------------------------------- bass prog
def _build_nc():
    nc = bacc.Bacc("TRN2", target_bir_lowering=False, debug=False,
                   num_devices=N_CORES)

    p = {}
    for s in (0, 1):
        p[f"wt{s}a"] = nc.declare_dram_parameter(
            f"wt{s}a", [128, (NKC // 2) * 2 * N_ORD], FP8, isOutput=False)
        p[f"wt{s}b"] = nc.declare_dram_parameter(
            f"wt{s}b", [128, (NKC // 2) * 2 * N_ORD], FP8, isOutput=False)
        p[f"vt{s}"] = nc.declare_dram_parameter(
            f"vt{s}", [128, NKC * 2 * RANK], FP8, isOutput=False)
        p[f"emt{s}"] = nc.declare_dram_parameter(
            f"emt{s}", [128, 2 * PRED_LEN], FP8, isOutput=False)
    p["ftx8"] = nc.declare_dram_parameter("ftx8", [128, SEQ_LEN], FP8,
                                          isOutput=False)
    p["u80"] = nc.declare_dram_parameter("u80", [128, 4 * RANK], FP8,
                                         isOutput=False)
    p["u81"] = nc.declare_dram_parameter("u81", [128, 8 * RANK], FP8,
                                         isOutput=False)
    p["xbt"] = nc.declare_dram_parameter("xbt", [128, SEQ_LEN], BF16,
                                         isOutput=False)
    p["negsu"] = nc.declare_dram_parameter("negsu", [1, 2 * RANK], BF16,
                                           isOutput=False)
    p["mlpwb"] = nc.declare_dram_parameter("mlpwb", [1, 3], F32,
                                           isOutput=False)
    p["out_dec"] = nc.declare_dram_parameter("out_dec", [BE, PRED_LEN],
                                             F16, isOutput=True)

    with tile.TileContext(nc, num_cores=N_CORES) as tc:
        _emit(nc, tc, p)
    nc.finalize()
    return nc


def _emit(nc, tc, p):
    AF = mybir.ActivationFunctionType
    DR = mybir.MatmulPerfMode.DoubleRow
    with ExitStack() as ctx:
        const = ctx.enter_context(tc.tile_pool(name="const", bufs=1))
        work = ctx.enter_context(tc.tile_pool(name="work", bufs=1))
        ps_p = ctx.enter_context(
            tc.tile_pool(name="ps_p", bufs=2, space="PSUM"))
        ps_q = ctx.enter_context(
            tc.tile_pool(name="ps_q", bufs=2, space="PSUM"))
        ps_gt = ctx.enter_context(
            tc.tile_pool(name="ps_gt", bufs=1, space="PSUM"))
        ps_tr = ctx.enter_context(
            tc.tile_pool(name="ps_tr", bufs=2, space="PSUM"))
        ps_dec = ctx.enter_context(
            tc.tile_pool(name="ps_dec", bufs=1, space="PSUM"))

        # ---- DMAs in consumption order --------------------------------
        # sync HWDGE: the w stream (P's critical path) + output store
        wt = {}
        for s in (0, 1):
            for hf, nm in ((0, "a"), (1, "b")):
                t = const.tile([128, NKC // 2, 2, N_ORD], FP8,
                               tag=f"wt{s}{nm}", name=f"wt{s}{nm}")
                nc.sync.dma_start(t[:], p[f"wt{s}{nm}"][:, :])
                wt[s, hf] = t
        # scalar HWDGE: everything else, small/early things first
        mlpwb_sb = const.tile([1, 3], F32, tag="mlpwb")
        nc.scalar.dma_start(mlpwb_sb[:], p["mlpwb"][:, :])
        negsu = const.tile([1, 2 * RANK], BF16, tag="negsu")
        nc.scalar.dma_start(negsu[:], p["negsu"][:, :])
        vt = {}
        vt[0] = const.tile([128, NKC, 2, RANK], FP8, tag="vt0", name="vt0")
        nc.scalar.dma_start(vt[0][:], p["vt0"][:, :])
        xbt = const.tile([128, SEQ_LEN], BF16, tag="xbt")
        nc.scalar.dma_start(xbt[:], p["xbt"][:, :])
        vt[1] = const.tile([128, NKC, 2, RANK], FP8, tag="vt1", name="vt1")
        nc.scalar.dma_start(vt[1][:], p["vt1"][:, :])
        fxu = const.tile([128, 4, 2, 128], FP8, tag="fxu")
        nc.scalar.dma_start(fxu[:], p["ftx8"][:, :])
        u = {}
        for s in (0, 1):
            lch = (s + 1) * 2
            u[s] = const.tile([128, lch, 2, RANK], FP8, tag=f"u{s}",
                              name=f"u{s}")
            nc.scalar.dma_start(u[s][:], p[f"u8{s}"][:, :])
        emt = {}
        for s in (0, 1):
            emt[s] = const.tile([128, 2, PRED_LEN], FP8, tag=f"emt{s}",
                                name=f"emt{s}")
            nc.scalar.dma_start(emt[s][:], p[f"emt{s}"][:, :])

        ident = const.tile([128, 128], BF16, tag="ident")
        make_identity(nc, ident[:])
        ident32 = const.tile([128, 128], F32, tag="ident32")
        make_identity(nc, ident32[:])
        ones = const.tile([1, 128], F32, tag="ones")
        nc.vector.memset(ones[:], 1.0)

        # ---- series stats on DVE/ACT (never on the PE critical path) ---
        sumx = work.tile([BE, 1], F32, tag="sumx")
        nc.vector.reduce_sum(sumx[:], xbt[:], axis=mybir.AxisListType.X)
        sumsq = work.tile([BE, 1], F32, tag="sumsq")
        sqd = work.tile([BE, SEQ_LEN], BF16, tag="sqd")
        nc.scalar.activation(sqd[:], xbt[:], AF.Square, accum_out=sumsq[:])
        mean = work.tile([BE, 1], F32, tag="mean")
        nc.vector.tensor_scalar_mul(mean[:], sumx[:], 1.0 / SEQ_LEN)
        ex2 = work.tile([BE, 1], F32, tag="ex2")
        nc.vector.tensor_scalar_mul(ex2[:], sumsq[:], 1.0 / SEQ_LEN)
        m2 = work.tile([BE, 1], F32, tag="m2")
        nc.vector.tensor_mul(m2[:], mean[:], mean[:])
        var = work.tile([BE, 1], F32, tag="var")
        nc.vector.tensor_sub(var[:], ex2[:], m2[:])
        eps = work.tile([BE, 1], F32, tag="eps")
        nc.vector.memset(eps[:], 1e-5)
        std = work.tile([BE, 1], F32, tag="std")
        nc.scalar.activation(std[:], var[:], AF.Sqrt, bias=eps[:])

        # ws broadcast: ws_sb[p, 0:2] = m_s descales, ws_sb[p, 2] = bias
        ps_w = ps_tr.tile([128, 3], F32, tag="tr", name="ps_w")
        nc.tensor.matmul(ps_w[:], lhsT=ones[:], rhs=mlpwb_sb[:])
        ws_sb = work.tile([128, 3], F32, tag="ws")
        nc.vector.tensor_copy(ws_sb[:], ps_w[:])

        # ---- P_s = V_s @ w_s : fp8 DoubleRow --------------------------
        p_sb = {}
        for s in (0, 1):
            p_ps = ps_p.tile([RANK, N_ORD], F32, tag="p", name=f"p{s}")
            for c in range(NKC):
                nc.tensor.matmul(
                    p_ps[:],
                    lhsT=vt[s][:, c, :, :],
                    rhs=wt[s, c // 4][:, c % 4, :, :],
                    perf_mode=DR,
                    start=(c == 0), stop=(c == NKC - 1))
            p_sb[s] = work.tile([RANK, N_ORD], BF16, tag=f"p_sb{s}",
                                name=f"p_sb{s}")
            nc.vector.tensor_copy(p_sb[s][:], p_ps[:])

        # transpose P (fp8) -> pt [o-part, j, r]
        pt_sb = {}
        for s in (0, 1):
            pt_sb[s] = work.tile([128, 2, RANK], FP8, tag=f"pt{s}",
                                 name=f"pt{s}")
            for c in (0, 1):
                tr = ps_tr.tile([128, RANK], BF16, tag="tr",
                                name=f"tr{s}{c}")
                nc.tensor.transpose(
                    tr[:], p_sb[s][:, c * 128:(c + 1) * 128],
                    ident[:RANK, :RANK])
                nc.vector.tensor_copy(pt_sb[s][:, c, :], tr[:])

        # Q_s = P_s @ EmT_s : one fp8 DoubleRow matmul per scale
        q_sb = {}
        for s in (0, 1):
            q_ps = ps_q.tile([RANK, PRED_LEN], F32, tag="q", name=f"q{s}")
            nc.tensor.matmul(q_ps[:], lhsT=pt_sb[s][:], rhs=emt[s][:],
                             perf_mode=DR, start=True, stop=True)
            q_sb[s] = work.tile([RANK, PRED_LEN], BF16, tag=f"q_sb{s}",
                                name=f"q_sb{s}")
            # fold mlp weight and all fp8 descales into Q
            nc.vector.tensor_scalar_mul(q_sb[s][:], q_ps[:],
                                        ws_sb[:RANK, s:s + 1])

        # mu as a bf16 row (for the rank-1 norm correction in gT)
        ps_mu = ps_tr.tile([1, 128], F32, tag="tr", name="ps_mu")
        nc.tensor.transpose(ps_mu[:], mean[:], ident32[:])
        mu_row = work.tile([1, 128], BF16, tag="mu_row")
        nc.vector.tensor_copy(mu_row[:], ps_mu[:])

        # gT_s = U_s.T @ f - SU_s x mu : fp8 DoubleRow over time chunks
        gt_sb = {}
        for s in (0, 1):
            lch = (s + 1) * 2
            j0 = 4 - lch
            gt_ps = ps_gt.tile([RANK, BE], F32, tag="gt", name=f"gt{s}")
            for d in range(lch):
                nc.tensor.matmul(gt_ps[:], lhsT=u[s][:, d, :, :],
                                 rhs=fxu[:, j0 + d, :, :],
                                 perf_mode=DR, start=(d == 0), stop=False)
            nc.tensor.matmul(gt_ps[:],
                             lhsT=negsu[:, s * RANK:(s + 1) * RANK],
                             rhs=mu_row[:], start=False, stop=True)
            gt_sb[s] = work.tile([RANK, BE], BF16, tag=f"gt_sb{s}",
                                 name=f"gt_sb{s}")
            nc.vector.tensor_copy(gt_sb[s][:], gt_ps[:])

        # dec[be, p] = sum_s gT_s.T @ Q_s, split in column halves so the
        # first half stores while the second is still accumulating
        bmu = work.tile([BE, 1], F32, tag="bmu")
        nc.vector.tensor_mul(bmu[:], ws_sb[:, 2:3], std[:])
        nc.vector.tensor_add(bmu[:], bmu[:], mean[:])
        bmu8 = work.tile([BE, 1], F32, tag="bmu8")
        nc.vector.tensor_scalar_mul(bmu8[:], bmu[:], 1.0 / N_CORES)

        dec_ps = ps_dec.tile([BE, PRED_LEN], F32, tag="dec")
        H = PRED_LEN // 2
        for h in (0, 1):
            for s in (0, 1):
                nc.tensor.matmul(dec_ps[:, h * H:(h + 1) * H],
                                 lhsT=gt_sb[s][:],
                                 rhs=q_sb[s][:, h * H:(h + 1) * H],
                                 start=(s == 0), stop=(s == 1))
            out_sb = work.tile([BE, H], F16, tag=f"out{h}", name=f"out{h}")
            nc.vector.tensor_scalar_add(out_sb[:],
                                        dec_ps[:, h * H:(h + 1) * H],
                                        bmu8[:])
            nc.sync.dma_start(p["out_dec"][:, h * H:(h + 1) * H],
                              out_sb[:])


_NC = None


def _get_nc():
    global _NC
    if _NC is None:
        _NC = _build_nc()
    return _NC


# ---------------------------------------------------------------- host side
def _pack_chunks(a, inner):
    """(n*256, inner) contraction-major -> [128, n, 2, inner] flat."""
    n = a.shape[0] // 256
    return np.ascontiguousarray(
        a.reshape(n, 2, 128, inner).transpose(2, 0, 1, 3).reshape(
            128, n * 2 * inner))


_CONST_MAPS = None


def _const_maps():
    """Input-independent packed constants."""
    global _CONST_MAPS
    if _CONST_MAPS is None:
        consts = _get_consts()
        shared = {}
        percore = [dict() for _ in range(N_CORES)]
        uscs, escs = [], []
        u_flat = []
        negsu = np.empty((1, 2 * RANK), np.float32)
        for s in (0, 1):
            U, Vn, EmT = consts[s]
            usc = 128.0 / np.abs(U).max()
            esc = 128.0 / np.abs(EmT).max()
            uscs.append(usc)
            escs.append(esc)
            # emt: [128 (o%128), 2 (o chunk), 512] in fp8, DoubleRow pairs
            shared[f"emt{s}"] = np.ascontiguousarray(
                _f8(EmT * esc).reshape(2, 128, PRED_LEN)
                .transpose(1, 0, 2).reshape(128, 2 * PRED_LEN))
            # u: DoubleRow over time, [128, lch, 2, r]
            u_dev = _f8(U * usc)                          # (L, r)
            negsu[0, s * RANK:(s + 1) * RANK] = (
                -FSC * u_dev.astype(np.float32).sum(0))
            u_flat.append(_pack_chunks(u_dev.astype(np.float32), RANK))
            for c in range(N_CORES):
                n0 = c * NSL
                cols = np.concatenate(
                    [np.arange(n0 * MODES, (n0 + NSL) * MODES),
                     N_ORD * MODES + np.arange(n0 * MODES,
                                               (n0 + NSL) * MODES)])
                percore[c][f"vt{s}"] = _f8(_pack_chunks(
                    np.ascontiguousarray(Vn[:, cols].T) * VSCK, RANK))
        shared["negsu"] = negsu.astype(BF16_NP)
        shared["_u80"] = _f8(u_flat[0])
        shared["_u81"] = _f8(u_flat[1])
        shared["_uscs"] = uscs
        shared["_escs"] = escs
        _CONST_MAPS = (shared, percore)
    return _CONST_MAPS


def _in_maps(x_enc, spec_w_real, spec_w_imag, mlp_weight, mlp_bias):
    shared_c, percore_c = _const_maps()
    uscs, escs = shared_c["_uscs"], shared_c["_escs"]
    # ftx: DoubleRow time pairs [128, 4, 2, 128]: t = c*256 + j*128 + p
    ftx = _pack_chunks(
        x_enc.transpose(1, 0, 2).reshape(SEQ_LEN, BE) * FSC, BE)
    xbt = np.ascontiguousarray(
        np.transpose(x_enc, (0, 2, 1)).reshape(BE, SEQ_LEN))

    mw = np.asarray(mlp_weight, np.float32).reshape(2)
    mlpwb = np.empty((1, 3), np.float32)
    wscs = []
    for s in (0, 1):
        wmax = max(np.abs(spec_w_real[s]).max(),
                   np.abs(spec_w_imag[s]).max(), 1e-30)
        wsc = 128.0 / wmax
        wscs.append(wsc)
        mlpwb[0, s] = mw[s] / (VSCK * wsc * escs[s] * uscs[s] * FSC)
    mlpwb[0, 2] = np.asarray(mlp_bias, np.float32).reshape(1)[0]

    shared = {"ftx8": _f8(ftx), "xbt": xbt.astype(BF16_NP),
              "u80": shared_c["_u80"], "u81": shared_c["_u81"],
              "emt0": shared_c["emt0"], "emt1": shared_c["emt1"],
              "negsu": shared_c["negsu"], "mlpwb": mlpwb}

    maps = []
    for c in range(N_CORES):
        n0 = c * NSL
        m = dict(shared)
        m.update(percore_c[c])
        for s in (0, 1):
            w_all = np.concatenate([
                spec_w_real[s, n0:n0 + NSL].transpose(0, 2, 1).reshape(
                    -1, N_ORD),
                spec_w_imag[s, n0:n0 + NSL].transpose(0, 2, 1).reshape(
                    -1, N_ORD)], axis=0) * wscs[s]       # (2048, 256)
            packed = _f8(_pack_chunks(w_all, N_ORD))
            half = (NKC // 2) * 2 * N_ORD
            m[f"wt{s}a"] = np.ascontiguousarray(packed[:, :half])
            m[f"wt{s}b"] = np.ascontiguousarray(packed[:, half:])
        maps.append(m)
    return maps


def kernel(x_enc, spec_w_real, spec_w_imag, mlp_weight, mlp_bias,
           _trace=False, _trace_kwargs=None):
    x_enc = np.asarray(x_enc, np.float32)
    spec_w_real = np.asarray(spec_w_real, np.float32)
    spec_w_imag = np.asarray(spec_w_imag, np.float32)
    maps = _in_maps(x_enc, spec_w_real, spec_w_imag, mlp_weight, mlp_bias)
    nc = _get_nc()
    res = run_bass_kernel_spmd(nc, maps, list(range(N_CORES)),
                               trace=_trace, **(_trace_kwargs or {}))
    # out_dec[c] = partial dec over core c's n-shard; unshard = sum
    full = np.sum([res.results[c]["out_dec"].astype(np.float32)
                   for c in range(N_CORES)], axis=0, dtype=np.float32)
    out = np.ascontiguousarray(
        full.reshape(B_SZ, E_IN, PRED_LEN).transpose(0, 2, 1), np.float32)
    if _trace:
        return out, res
    return out
